# revision 7
# baseline (speedup 1.0000x reference)
"""Causal RCNN (causal conv1d -> LayerNorm -> SRU scan -> lambda-mix -> RMSNorm)
Trainium2 Bass kernel.

Sharding: data-parallel over batch B=16 across 8 NeuronCores (2 batches/core).
On-chip layout: feature-major [d on partitions (8 chunks x 128), (b, l) free].
Matmuls run in float32r (1 cycle/row at N>=256). The SRU recurrence
c_t = f_t*c_{t-1} + (1-f_t)*u0_t with f_t = sigmoid(u1_t + v_f*c_{t-1} + b_f)
is solved by fixed-point iteration (KS rounds) where each round evaluates the
gates in bulk and runs the linear recurrence with the hardware
tensor_tensor_scan instruction. |v_f| <= 0.18 makes the gate's c-dependence a
contraction (~1/14 per round); KS=4 reaches ~3e-5 abs error, below the
float32r matmul rounding noise.
"""

import numpy as np

import concourse.bacc as bacc
import concourse.tile as tile
from concourse import mybir, bass_utils
from concourse.bass_interp import get_hw_module
from concourse.mybir import ActivationFunctionType as actf
from concourse.mybir import AluOpType as alu

B, L, D, KK = 16, 1024, 1024, 3
NCORES = 8
BL = B // NCORES      # batches per core
TC = 256              # l-chunk length
NCH = L // TC
KS = 4                # scan fixed-point iterations
DC = D // 128         # d-chunks
EC = 3 * D // 128     # e-chunks of U
EPS_LN, EPS_RMS = 1e-5, 1e-6
F32, F32R = mybir.dt.float32, mybir.dt.float32r

# pp param slots (per-partition constants, laid out [128, DC, NPP])
VF, VR, LAM, RMSW, CB, LNG, LNB = range(7)
NPP = 8


def _build_tile(tc_, yfm, xfm, convw, sruw, ubcol_d, pp_d, zz_d, apply_affine):
    nc = tc_.nc
    ctxpools = []

    def pool(name, bufs, space="SBUF"):
        p = tc_.alloc_tile_pool(name=name, bufs=bufs, space=space)
        ctxpools.append(p)
        return p

    singles = pool("singles", 1)
    cwp = pool("cw", 4)
    swp = pool("sw", 2)
    xp_p = pool("xp", 2)
    cnnp = pool("cnn", 1)
    nrmp = pool("nrm", 1)
    uap = pool("ua", 1)
    ubp = pool("ub", 1)
    ucp = pool("uc", 1)
    cp = pool("c", 1)
    zfp = pool("zf", 1)
    gp_ = pool("g", 1)
    yp = pool("y", 2)
    stp = pool("st", 4)
    bcp = pool("bc", 2)
    carryp = pool("carry", 2)
    xtp = pool("xt", 1) if apply_affine else None

    pconv = pool("pconv", 2, space="PSUM")
    pu = pool("pu", 2, space="PSUM")
    pstat = pool("pstat", 2, space="PSUM")
    pbc = pool("pbc", 2, space="PSUM")

    # constants / params
    pp = singles.tile([128, DC, NPP], F32)
    nc.sync.dma_start(pp, pp_d)
    ubcol = singles.tile([128, EC], F32)
    nc.sync.dma_start(ubcol, ubcol_d)
    onec32 = singles.tile([128, 1], F32)
    nc.vector.memset(onec32, 1.0)
    onec_r = singles.tile([128, 1], F32R)
    nc.vector.tensor_copy(onec_r, onec32)
    oner32 = singles.tile([1, 128], F32)
    nc.vector.memset(oner32, 1.0)
    eps_ln_t = singles.tile([1, 1], F32)
    nc.vector.memset(eps_ln_t, EPS_LN)
    eps_rms_t = singles.tile([1, 1], F32)
    nc.vector.memset(eps_rms_t, EPS_RMS)

    carry_prev = None

    for ci in range(NCH):
        l0 = ci * TC
        # ---- x chunk load (feature-major, with left halo of 2) ----
        xp = xp_p.tile([128, DC, BL, TC + 2], F32R, tag="xp")
        for dc in range(DC):
            rows = slice(dc * 128, (dc + 1) * 128)
            for b in range(BL):
                if ci == 0:
                    nc.sync.dma_start(xp[:, dc, b, 0:2], zz_d.bitcast(F32R))
                    nc.sync.dma_start(xp[:, dc, b, 2:TC + 2],
                                      xfm[b, rows, 0:TC].bitcast(F32R))
                else:
                    nc.sync.dma_start(xp[:, dc, b, :],
                                      xfm[b, rows, l0 - 2:l0 + TC].bitcast(F32R))

        # ---- causal conv (3 shifted matmuls, bias via ACT Identity) ----
        cnn = cnnp.tile([128, DC, BL, TC], F32R, tag="cnn")
        for oc in range(DC):
            cws = []
            for k in range(KK):
                cw = cwp.tile([128, DC, 128], F32R, tag="cw")
                nc.sync.dma_start(cw, convw[oc, :, k].bitcast(F32R))
                cws.append(cw)
            ps = pconv.tile([128, BL, TC], F32, tag="pconv")
            n_mm = KK * DC
            i = 0
            for k in range(KK):
                for ic in range(DC):
                    nc.tensor.matmul(ps, cws[k][:, ic, :], xp[:, ic, :, k:k + TC],
                                     start=(i == 0), stop=(i == n_mm - 1))
                    i += 1
            nc.scalar.activation(cnn[:, oc], ps, actf.Identity,
                                 bias=pp[:, oc, CB:CB + 1], scale=1.0)

        # ---- LayerNorm over d (PE reductions + PE broadcast) ----
        cnn32 = cnn.bitcast(F32)
        sq = zfp.tile([128, DC, BL, TC], F32R, tag="zf")
        nc.scalar.activation(sq, cnn32, actf.Square)
        ps_sum = pstat.tile([1, BL, TC], F32, tag="pstat")
        for dc in range(DC):
            nc.tensor.matmul(ps_sum, onec_r, cnn[:, dc],
                             start=(dc == 0), stop=(dc == DC - 1))
        ps_sq = pstat.tile([1, BL, TC], F32, tag="pstat")
        for dc in range(DC):
            nc.tensor.matmul(ps_sq, onec_r, sq[:, dc],
                             start=(dc == 0), stop=(dc == DC - 1))
        mu = stp.tile([1, BL, TC], F32, tag="st")
        nc.vector.tensor_scalar(mu, ps_sum, 1.0 / D, None, op0=alu.mult)
        var = stp.tile([1, BL, TC], F32, tag="st")
        # var = E[x^2] - mu^2 = ps_sq/D - mu*mu
        nc.vector.tensor_scalar(var, ps_sq, 1.0 / D, None, op0=alu.mult)
        mu2 = stp.tile([1, BL, TC], F32, tag="st")
        nc.vector.tensor_tensor(mu2, mu, mu, op=alu.mult)
        nc.vector.tensor_tensor(var, var, mu2, op=alu.subtract)
        sd = stp.tile([1, BL, TC], F32, tag="st")
        nc.scalar.activation(sd, var, actf.Sqrt, bias=eps_ln_t)
        rstd = stp.tile([1, BL, TC], F32, tag="st")
        nc.vector.reciprocal(rstd, sd)
        nm = stp.tile([1, BL, TC], F32, tag="st")
        nc.vector.scalar_tensor_tensor(nm, mu, -1.0, rstd,
                                       op0=alu.mult, op1=alu.mult)
        # broadcasts across partitions via K=1 fp32 matmuls (exact)
        pb_rs = pbc.tile([128, BL, TC], F32, tag="pbc")
        nc.tensor.matmul(pb_rs, oner32, rstd, start=True, stop=True)
        pb_nm = pbc.tile([128, BL, TC], F32, tag="pbc")
        nc.tensor.matmul(pb_nm, oner32, nm, start=True, stop=True)
        rs_sb = bcp.tile([128, BL, TC], F32, tag="bc")
        nc.vector.tensor_copy(rs_sb, pb_rs)
        # normed = cnn*rstd - mu*rstd : gp mult (SBUF), DVE add (PSUM)
        ttile = gp_.tile([128, DC, BL, TC], F32, tag="g")
        nrm = nrmp.tile([128, DC, BL, TC], F32R, tag="nrm")
        for dc in range(DC):
            nc.gpsimd.tensor_tensor(ttile[:, dc], cnn32[:, dc], rs_sb, op=alu.mult)
        for dc in range(DC):
            nc.vector.tensor_tensor(nrm[:, dc], ttile[:, dc], pb_nm, op=alu.add)
        nrm32 = nrm.bitcast(F32)
        if apply_affine:
            xt = xtp.tile([128, DC, BL, TC], F32, tag="xt")
            for dc in range(DC):
                nc.vector.tensor_scalar(xt[:, dc], nrm32[:, dc],
                                        pp[:, dc, LNG:LNG + 1],
                                        pp[:, dc, LNB:LNB + 1],
                                        op0=alu.mult, op1=alu.add)
        else:
            xt = nrm32

        # ---- U = normed @ sru_w_eff (+u_bias via ACT Identity copy) ----
        u0 = uap.tile([128, DC, BL, TC], F32, tag="ua")
        u1b = ubp.tile([128, DC, BL, TC], F32, tag="ub")
        u2b = ucp.tile([128, DC, BL, TC], F32, tag="uc")
        utiles = [u0, u1b, u2b]
        for ec in range(EC):
            j, e8 = divmod(ec, DC)
            sw = swp.tile([128, DC, 128], F32R, tag="sw")
            nc.sync.dma_start(sw, sruw[ec].bitcast(F32R))
            psu = pu.tile([128, BL, TC], F32, tag="pu")
            for dc in range(DC):
                nc.tensor.matmul(psu, sw[:, dc, :], nrm[:, dc],
                                 start=(dc == 0), stop=(dc == DC - 1))
            nc.scalar.activation(utiles[j][:, e8], psu, actf.Identity,
                                 bias=ubcol[:, ec:ec + 1], scale=1.0)

        # ---- SRU scan: KS fixed-point rounds ----
        c = cp.tile([128, DC, BL, TC + 1], F32, tag="c")
        if ci == 0:
            nc.vector.memset(c[:, :, :, 0:1], 0.0)
        else:
            nc.vector.tensor_copy(c[:, :, :, 0:1], carry_prev)
        zt = zfp.tile([128, DC, BL, TC], F32, tag="zf")
        gt = gp_.tile([128, DC, BL, TC], F32, tag="g")
        for it in range(KS):
            for dc in range(DC):
                for b in range(BL):
                    nc.vector.scalar_tensor_tensor(
                        zt[:, dc, b, :], c[:, dc, b, 0:TC],
                        pp[:, dc, VF:VF + 1], u1b[:, dc, b, :],
                        op0=alu.mult, op1=alu.add)
            for b in range(BL):
                nc.scalar.activation(gt[:, :, b, :], zt[:, :, b, :],
                                     actf.Sigmoid, scale=-1.0)   # 1-f
                nc.scalar.activation(zt[:, :, b, :], zt[:, :, b, :],
                                     actf.Sigmoid)               # f (in place)
            for b in range(BL):
                nc.gpsimd.tensor_tensor(gt[:, :, b, :], gt[:, :, b, :],
                                        u0[:, :, b, :], op=alu.mult)
            for dc in range(DC):
                for b in range(BL):
                    nc.vector.tensor_tensor_scan(
                        c[:, dc, b, 1:TC + 1], zt[:, dc, b, :], gt[:, dc, b, :],
                        c[:, dc, b, 0:1], op0=alu.mult, op1=alu.add)
        carry = carryp.tile([128, DC, BL, 1], F32, tag="carry")
        nc.vector.tensor_copy(carry, c[:, :, :, TC:TC + 1])
        carry_prev = carry

        # ---- post: r, tanh, highway, lambda mix ----
        # z2 = vr*c_{t-1} + u2b -> zt ; r = sigmoid(z2) in place
        for dc in range(DC):
            for b in range(BL):
                nc.vector.scalar_tensor_tensor(
                    zt[:, dc, b, :], c[:, dc, b, 0:TC],
                    pp[:, dc, VR:VR + 1], u2b[:, dc, b, :],
                    op0=alu.mult, op1=alu.add)
        for b in range(BL):
            nc.scalar.activation(zt[:, :, b, :], zt[:, :, b, :], actf.Sigmoid)
        # th = tanh(c_t) -> u0 (overwrite); s = th - xt; m = r*s; h = m + xt
        for b in range(BL):
            nc.scalar.activation(u0[:, :, b, :], c[:, :, b, 1:TC + 1], actf.Tanh)
        for b in range(BL):
            nc.gpsimd.tensor_tensor(u0[:, :, b, :], u0[:, :, b, :],
                                    xt[:, :, b, :], op=alu.subtract)
        for b in range(BL):
            nc.vector.tensor_tensor(u0[:, :, b, :], zt[:, :, b, :],
                                    u0[:, :, b, :], op=alu.mult)
        for b in range(BL):
            nc.gpsimd.tensor_tensor(u0[:, :, b, :], u0[:, :, b, :],
                                    xt[:, :, b, :], op=alu.add)
        # q = cnn - h -> gt ; out_pre = q*lam + h -> cnn tile (f32r)
        for b in range(BL):
            nc.gpsimd.tensor_tensor(gt[:, :, b, :], cnn32[:, :, b, :],
                                    u0[:, :, b, :], op=alu.subtract)
        for dc in range(DC):
            nc.vector.scalar_tensor_tensor(
                cnn[:, dc], gt[:, dc], pp[:, dc, LAM:LAM + 1], u0[:, dc],
                op0=alu.mult, op1=alu.add)

        # ---- RMSNorm over d ----
        sq2 = gp_.tile([128, DC, BL, TC], F32R, tag="g")
        nc.scalar.activation(sq2, cnn32, actf.Square)
        ps2 = pstat.tile([1, BL, TC], F32, tag="pstat")
        for dc in range(DC):
            nc.tensor.matmul(ps2, onec_r, sq2[:, dc],
                             start=(dc == 0), stop=(dc == DC - 1))
        ms = stp.tile([1, BL, TC], F32, tag="st")
        nc.vector.tensor_scalar(ms, ps2, 1.0 / D, None, op0=alu.mult)
        sd2 = stp.tile([1, BL, TC], F32, tag="st")
        nc.scalar.activation(sd2, ms, actf.Sqrt, bias=eps_rms_t)
        rstd2 = stp.tile([1, BL, TC], F32, tag="st")
        nc.vector.reciprocal(rstd2, sd2)
        pb2 = pbc.tile([128, BL, TC], F32, tag="pbc")
        nc.tensor.matmul(pb2, oner32, rstd2, start=True, stop=True)
        rs2_sb = bcp.tile([128, BL, TC], F32, tag="bc")
        nc.vector.tensor_copy(rs2_sb, pb2)
        # y = (out_pre * rms_w) * rstd2
        fy = zfp.tile([128, DC, BL, TC], F32, tag="zf")
        for dc in range(DC):
            nc.scalar.activation(fy[:, dc], cnn32[:, dc], actf.Copy,
                                 scale=pp[:, dc, RMSW:RMSW + 1])
        for dc in range(DC):
            yo = yp.tile([128, BL, TC], F32, tag="y")
            nc.gpsimd.tensor_tensor(yo, fy[:, dc], rs2_sb, op=alu.mult)
            rows = slice(dc * 128, (dc + 1) * 128)
            for b in range(BL):
                nc.sync.dma_start(yfm[b, rows, l0:l0 + TC], yo[:, b, :])

    for p in reversed(ctxpools):
        p.release()


_NC_CACHE = {}


def _get_nc(apply_affine):
    key = bool(apply_affine)
    if key not in _NC_CACHE:
        nc = bacc.Bacc("TRN2", target_bir_lowering=False, debug=False,
                       num_devices=NCORES)
        xfm = nc.dram_tensor("xfm", (BL, D, L), F32, kind="ExternalInput").ap()
        convw = nc.dram_tensor("convw", (DC, 128, KK, DC, 128), F32,
                               kind="ExternalInput").ap()
        sruw = nc.dram_tensor("sruw", (EC, 128, DC, 128), F32,
                              kind="ExternalInput").ap()
        ubcol = nc.dram_tensor("ubcol", (128, EC), F32, kind="ExternalInput").ap()
        pp_d = nc.dram_tensor("pp", (128, DC, NPP), F32, kind="ExternalInput").ap()
        zz_d = nc.dram_tensor("zz", (128, 2), F32, kind="ExternalInput").ap()
        yfm = nc.dram_tensor("yfm", (BL, D, L), F32, kind="ExternalOutput").ap()
        with tile.TileContext(nc) as tc_:
            _build_tile(tc_, yfm, xfm, convw, sruw, ubcol, pp_d, zz_d, key)
        nc.compile()
        nc.m = get_hw_module(nc.m)
        _NC_CACHE[key] = nc
    return _NC_CACHE[key]


def _prep_shared(conv_w, conv_b, ln_g, ln_b, sru_w, sru_v, sru_b,
                 lambda_w, rms_w):
    convw_sb = np.ascontiguousarray(
        conv_w.reshape(DC, 128, DC, 128, KK).transpose(0, 3, 4, 2, 1))
    sru_w_r = np.ascontiguousarray(
        sru_w.reshape(D, D, 3).transpose(0, 2, 1)).reshape(D, 3 * D)
    sru_w_eff = np.ascontiguousarray(ln_g[:, None] * sru_w_r, dtype=np.float32)
    u_bias = (ln_b.astype(np.float64) @ sru_w_r.astype(np.float64)).astype(np.float32)
    u_bias[D:2 * D] += sru_b[:D]
    u_bias[2 * D:] += sru_b[D:]
    sruw_sb = np.ascontiguousarray(
        sru_w_eff.reshape(DC, 128, EC, 128).transpose(2, 1, 0, 3))
    ubcol = np.ascontiguousarray(u_bias.reshape(EC, 128).T)
    pp = np.zeros((128, DC, NPP), np.float32)
    pp[:, :, VF] = sru_v[:D].reshape(DC, 128).T
    pp[:, :, VR] = sru_v[D:].reshape(DC, 128).T
    pp[:, :, LAM] = lambda_w.reshape(DC, 128).T
    pp[:, :, RMSW] = rms_w.reshape(DC, 128).T
    pp[:, :, CB] = conv_b.reshape(DC, 128).T
    pp[:, :, LNG] = ln_g.reshape(DC, 128).T
    pp[:, :, LNB] = ln_b.reshape(DC, 128).T
    return convw_sb, sruw_sb, ubcol, pp


def kernel(**inputs):
    f32 = lambda k: np.ascontiguousarray(np.asarray(inputs[k]), dtype=np.float32)
    x = f32("x")
    conv_w, conv_b = f32("conv_w"), f32("conv_b")
    ln_g, ln_b = f32("ln_g"), f32("ln_b")
    sru_w, sru_v, sru_b = f32("sru_w"), f32("sru_v"), f32("sru_b")
    lambda_w, rms_w = f32("lambda_w"), f32("rms_w")

    apply_affine = not (np.all(ln_g == 1.0) and np.all(ln_b == 0.0))
    nc = _get_nc(apply_affine)
    convw_sb, sruw_sb, ubcol, pp = _prep_shared(
        conv_w, conv_b, ln_g, ln_b, sru_w, sru_v, sru_b, lambda_w, rms_w)
    zz = np.zeros((128, 2), np.float32)

    in_maps = []
    for c in range(NCORES):
        xfm = np.ascontiguousarray(x[BL * c:BL * (c + 1)].transpose(0, 2, 1))
        in_maps.append(dict(xfm=xfm, convw=convw_sb, sruw=sruw_sb,
                            ubcol=ubcol, pp=pp, zz=zz))
    res = bass_utils.run_bass_kernel_spmd(nc, in_maps,
                                          core_ids=list(range(NCORES)))
    y = np.empty((B, L, D), np.float32)
    for c in range(NCORES):
        y[BL * c:BL * (c + 1)] = res.results[c]["yfm"].transpose(0, 2, 1)
    return y
